# revision 1
# baseline (speedup 1.0000x reference)
"""Trilinear SDF grid interpolation on 8 Trainium2 NeuronCores.

Strategy:
  - Host packs the 256^3 grid into an 8-corner struct table: ptab[cell] =
    the 2x2x2 corner values of cell (32B). One indirect-DMA descriptor then
    fetches all 8 corners of a query point at once.
  - Query points are sharded across 8 cores (250,112 padded points each),
    laid out [3, 128, T] so each partition owns T points.
  - On device (per core): regular-grid searchsorted is pure arithmetic --
    u=(c+0.64)*200, i0=round(u), then a +-1 correction against exactly
    recomputed grid coordinates (device fp32 matches host fp32 bit-exactly).
    Weights/denominator per reference semantics; one gather per 128 points
    ([128,1] offsets -> [128,8] dest, the only offset shape the DynamicDMA
    lowering handles correctly); weighted sum via an interleaved weight tile
    and a last-axis reduce.
"""
import numpy as np

GRID = 256
SCALE = 0.005
OFFSET = -0.64
NCORES = 8
P = 128
K = 2_000_000
T = 1954                     # point-slots per partition per core
PER_CORE = P * T             # 250,112
CHUNK = 256                  # slots per compute chunk (SBUF-bounded)

_cache = {}


def _build(nc_T):
    import concourse.bacc as bacc
    import concourse.bass as bass
    import concourse.mybir as mybir
    import concourse.tile as tile

    f32 = mybir.dt.float32
    i32 = mybir.dt.int32
    Alu = mybir.AluOpType

    nc = bacc.Bacc("TRN2", target_bir_lowering=False)
    xt = nc.dram_tensor("xt", [3, P, nc_T], f32, kind="ExternalInput")
    ptab = nc.dram_tensor("ptab", [GRID * GRID * GRID, 8], f32, kind="ExternalInput")
    out = nc.dram_tensor("out", [P, nc_T], f32, kind="ExternalOutput")

    chunks = []
    t0 = 0
    while t0 < nc_T:
        chunks.append((t0, min(CHUNK, nc_T - t0)))
        t0 += CHUNK

    with tile.TileContext(nc) as tc:
        with tc.tile_pool(name="sbuf", bufs=2) as pool:
            for (t0, C) in chunks:
                # ---- load coordinates [128, C] per axis ----
                cs = []
                for d in range(3):
                    ct = pool.tile([P, C], f32, tag=f"c{d}")
                    nc.sync.dma_start(out=ct[:], in_=xt[d, :, t0:t0 + C])
                    cs.append(ct)

                # ---- per-axis index math ----
                ils, dls, drs, os_ = [], [], [], []
                for d in range(3):
                    c = cs[d]
                    u = pool.tile([P, C], f32, tag=f"u{d}")
                    nc.vector.tensor_scalar(u[:], c[:], 0.64, 200.0, Alu.add, Alu.mult)
                    i0i = pool.tile([P, C], i32, tag=f"i0i{d}")
                    nc.vector.tensor_copy(i0i[:], u[:])          # f32->i32 (rne)
                    i0f = pool.tile([P, C], f32, tag=f"i0f{d}")
                    nc.vector.tensor_copy(i0f[:], i0i[:])        # exact back-convert
                    pa = pool.tile([P, C], f32, tag=f"pa{d}")
                    nc.vector.tensor_scalar(pa[:], i0f[:], 0.005, -0.64, Alu.mult, Alu.add)
                    pb = pool.tile([P, C], f32, tag=f"pb{d}")
                    nc.vector.tensor_scalar(pb[:], i0f[:], 1.0, 0.005, Alu.add, Alu.mult)
                    nc.vector.tensor_scalar(pb[:], pb[:], -0.64, None, Alu.add)
                    a = pool.tile([P, C], f32, tag=f"a{d}")
                    nc.vector.tensor_tensor(out=a[:], in0=pa[:], in1=c[:], op=Alu.is_lt)
                    b = pool.tile([P, C], f32, tag=f"b{d}")
                    nc.vector.tensor_tensor(out=b[:], in0=pb[:], in1=c[:], op=Alu.is_lt)
                    ilf = pool.tile([P, C], f32, tag=f"il{d}")
                    nc.vector.scalar_tensor_tensor(
                        out=ilf[:], in0=a[:], scalar=-1.0, in1=b[:],
                        op0=Alu.add, op1=Alu.add)
                    nc.vector.tensor_tensor(out=ilf[:], in0=ilf[:], in1=i0f[:], op=Alu.add)
                    nc.vector.tensor_scalar(ilf[:], ilf[:], 0.0, 254.0, Alu.max, Alu.min)
                    p_il = pa  # reuse
                    nc.vector.tensor_scalar(p_il[:], ilf[:], 0.005, -0.64, Alu.mult, Alu.add)
                    p_ir = pb  # reuse
                    nc.vector.tensor_scalar(p_ir[:], ilf[:], 1.0, 0.005, Alu.add, Alu.mult)
                    nc.vector.tensor_scalar(p_ir[:], p_ir[:], -0.64, None, Alu.add)
                    dl = a  # reuse
                    nc.vector.tensor_tensor(out=dl[:], in0=c[:], in1=p_il[:], op=Alu.subtract)
                    dr = b  # reuse
                    nc.vector.tensor_tensor(out=dr[:], in0=p_ir[:], in1=c[:], op=Alu.subtract)
                    o = pool.tile([P, C], f32, tag=f"o{d}")
                    nc.vector.tensor_tensor(out=o[:], in0=dl[:], in1=dr[:], op=Alu.add)
                    ils.append(ilf); dls.append(dl); drs.append(dr); os_.append(o)

                # ---- flat cell index (exact in fp32, < 2^24) ----
                idxf = pool.tile([P, C], f32, tag="idxf")
                nc.vector.tensor_scalar(idxf[:], ils[0][:], 65536.0, None, Alu.mult)
                nc.vector.scalar_tensor_tensor(
                    out=idxf[:], in0=ils[1][:], scalar=256.0, in1=idxf[:],
                    op0=Alu.mult, op1=Alu.add)
                nc.vector.tensor_tensor(out=idxf[:], in0=idxf[:], in1=ils[2][:], op=Alu.add)
                idxi = pool.tile([P, C], i32, tag="idxi")
                nc.vector.tensor_copy(idxi[:], idxf[:])

                # ---- gather packed corners: one [128,1] indirect DMA per slot ----
                g = pool.tile([P, C, 8], f32, tag="g")
                for t in range(C):
                    nc.gpsimd.indirect_dma_start(
                        out=g[:, t, :], out_offset=None,
                        in_=ptab[:],
                        in_offset=bass.IndirectOffsetOnAxis(ap=idxi[:, t:t + 1], axis=0),
                    )

                # ---- corner weights, interleaved [128, C, 8] ----
                # corner c = bx*4 + by*2 + bz ; weight = wx[bx]*wy[by]*wz[bz]
                # wx[0]=drx (left corner gets right distance), wx[1]=dlx
                w = pool.tile([P, C, 8], f32, tag="w")
                tyz = []
                for by in range(2):
                    for bz in range(2):
                        tt = pool.tile([P, C], f32, tag=f"tyz{by}{bz}")
                        wy = dls[1] if by else drs[1]
                        wz = dls[2] if bz else drs[2]
                        nc.vector.tensor_tensor(out=tt[:], in0=wy[:], in1=wz[:], op=Alu.mult)
                        tyz.append(tt)
                for cidx in range(8):
                    bx, byz = cidx >> 2, cidx & 3
                    wx = dls[0] if bx else drs[0]
                    nc.vector.tensor_tensor(
                        out=w[:, :, cidx], in0=tyz[byz][:], in1=wx[:], op=Alu.mult)

                # ---- weighted sum + denominator ----
                nc.vector.tensor_tensor(out=g[:, :, :], in0=g[:, :, :], in1=w[:, :, :],
                                        op=Alu.mult)
                num = pool.tile([P, C], f32, tag="num")
                nc.vector.tensor_reduce(num[:], g[:, :, :], mybir.AxisListType.X, Alu.add)
                den = pool.tile([P, C], f32, tag="den")
                nc.vector.tensor_tensor(out=den[:], in0=os_[0][:], in1=os_[1][:], op=Alu.mult)
                nc.vector.tensor_tensor(out=den[:], in0=den[:], in1=os_[2][:], op=Alu.mult)
                rcp = pool.tile([P, C], f32, tag="rcp")
                nc.vector.reciprocal(rcp[:], den[:])
                res = pool.tile([P, C], f32, tag="res")
                nc.vector.tensor_tensor(out=res[:], in0=num[:], in1=rcp[:], op=Alu.mult)
                nc.sync.dma_start(out=out[:, t0:t0 + C], in_=res[:])

    nc.compile()
    return nc


def _get_nc(nc_T):
    if nc_T not in _cache:
        _cache[nc_T] = _build(nc_T)
    return _cache[nc_T]


def _pack_table(values):
    v = np.ascontiguousarray(values, dtype=np.float32)
    packed = np.zeros((GRID, GRID, GRID, 8), np.float32)
    for bx in range(2):
        for by in range(2):
            for bz in range(2):
                c = bx * 4 + by * 2 + bz
                src = v[bx:, by:, bz:]
                packed[:src.shape[0], :src.shape[1], :src.shape[2], c] = src
    return packed.reshape(GRID * GRID * GRID, 8)


LAST_RESULTS = None


def kernel(x, values, px, py, pz, _T=T, _ncores=NCORES, _trace=False):
    global LAST_RESULTS
    from concourse import bass_utils

    x = np.ascontiguousarray(np.asarray(x), dtype=np.float32)
    k = x.shape[0]
    per_core = P * _T
    total = per_core * _ncores

    packed = _pack_table(np.asarray(values))

    xp = np.zeros((total, 3), np.float32)
    xp[:k] = x
    # core c, slot t, partition p  <- point c*per_core + t*128 + p
    xl = xp.reshape(_ncores, _T, P, 3).transpose(0, 3, 2, 1)  # [cores, 3, P, T]
    xl = np.ascontiguousarray(xl)

    nc = _get_nc(_T)
    in_maps = [{"xt": xl[c], "ptab": packed} for c in range(_ncores)]
    res = bass_utils.run_bass_kernel_spmd(
        nc, in_maps, core_ids=list(range(_ncores)), trace=_trace)
    LAST_RESULTS = res
    outs = [r["out"] for r in res.results]          # each [P, T]
    full = np.concatenate([o.T.reshape(-1) for o in outs])  # point order
    return np.ascontiguousarray(full[:k].astype(np.float32))

